# revision 1
# baseline (speedup 1.0000x reference)
"""LeViT-style attention block kernel for Trainium2 (8 NeuronCores, data-parallel over batch).

Reference computation (per batch b of 256, N=196 tokens, DIM=384):
  qkv = x @ qkv_w.T + qkv_b                      [196, 1152]
  q,k,v per head h (6): q,k [196,32], v [196,128]
  S = q @ k.T * 32^-0.5 + bias_h                 [196, 196]
  P = softmax(S, -1)
  O = P @ v  (concat heads -> [196, 768])
  A = hardswish(O)
  out = A @ proj_w.T + proj_b                    [196, 384]

Device mapping (per core: 32 batches = 6272 tokens):
  - host prepacks x.T (bf16), weight tiles, exp(bias) tables
  - qk computed transposed ([head-packed 128 rows, tokens]) on PE
  - v computed natural per batch ([tokens, 768])
  - S natural [n, m] -> exp on ACT -> *exp(bias) with fused row-sum on DVE
    -> P.T via PE matmul against diag(1/den) (transpose + normalize in one)
    -> O.T = v.T @ P.T on PE -> hardswish (ACT relu + fused min*mul on DVE)
    -> A.T accumulated into 128-token chunk tiles -> proj matmul -> out.
"""

import os
import sys

import numpy as np

sys.path.insert(0, "/opt/trn_rl_repo")

import ml_dtypes  # noqa: E402

import concourse.bass as bass  # noqa: E402
import concourse.tile as tile  # noqa: E402
from concourse import bacc, mybir  # noqa: E402
from concourse.bass_utils import run_bass_kernel_spmd  # noqa: E402

BF16 = mybir.dt.bfloat16
F32 = mybir.dt.float32
NPBF16 = ml_dtypes.bfloat16

N_CORES = 8
B, N, DIM = 256, 196, 384
KD, NH, D = 32, 6, 128  # key dim, heads, per-head v dim
DH = D * NH  # 768
RES = 14
SCALE = KD ** -0.5

AF = mybir.ActivationFunctionType
OP = mybir.AluOpType

# per-batch n/m tiling: token rows split 128 + 68
NT = [(0, 128), (128, 68)]

LAST_RESULT = {}  # test harness peeks at timing info here

# CoreSim rejects reads of never-written PSUM regions; the merged single-call
# exp / P.T-copy read (and discard) such garbage. EXACT_RANGES=True emits
# per-region calls instead — numerically identical — for simulator validation.
EXACT_RANGES = False
STAGES = 5  # debug bisect: 1=qkv only, 2=+S/exp/ttr, 3=+diag/PT, 4=+AV/hswish, 5=full


def _build_nc(bc):
    """Build the single-core program for bc batches (bc*196 tokens)."""
    T = bc * N
    assert T % 392 == 0
    nch = T // 392

    nc = bacc.Bacc("TRN2", target_bir_lowering=False, debug=False, num_devices=N_CORES)

    xt_d = nc.dram_tensor("xt", [DIM, T], BF16, kind="ExternalInput")
    wqk_d = nc.dram_tensor("wqk", [4, DIM, 128], BF16, kind="ExternalInput")
    bqk_d = nc.dram_tensor("bqk", [128, 4], F32, kind="ExternalInput")
    wv_d = nc.dram_tensor("wv", [DIM, DH], BF16, kind="ExternalInput")
    vb_d = nc.dram_tensor("vb", [128, DH], F32, kind="ExternalInput")
    wp_d = nc.dram_tensor("wp", [DH, DIM], BF16, kind="ExternalInput")
    pb_d = nc.dram_tensor("pb", [128, DIM], F32, kind="ExternalInput")
    pbr_d = nc.dram_tensor("pbr", [1, DIM], BF16, kind="ExternalInput")
    one_d = nc.dram_tensor("ones", [1, 128], BF16, kind="ExternalInput")
    eb_d = nc.dram_tensor("eb", [128, NH, 392], BF16, kind="ExternalInput")
    id_d = nc.dram_tensor("idm", [128, 128], BF16, kind="ExternalInput")
    hc_d = nc.dram_tensor("hc", [128, 1], F32, kind="ExternalInput")
    out_d = nc.dram_tensor("out", [T, DIM], F32, kind="ExternalOutput")

    with tile.TileContext(nc) as tc:
        with (
            tc.tile_pool(name="const", bufs=1) as cpool,
            tc.tile_pool(name="qkt", bufs=1) as qpool,
            tc.tile_pool(name="vtile", bufs=6) as vpool,
            tc.tile_pool(name="pu", bufs=3) as pupool,
            tc.tile_pool(name="pb2", bufs=8) as pbpool,
            tc.tile_pool(name="pts", bufs=3) as ptspool,
            tc.tile_pool(name="dg", bufs=3) as dgpool,
            tc.tile_pool(name="den", bufs=6) as denpool,
            tc.tile_pool(name="rt", bufs=3) as rpool,
            tc.tile_pool(name="atc", bufs=4) as atpool,
            tc.tile_pool(name="ob", bufs=3) as opool,
            tc.tile_pool(name="mmps", bufs=2, space="PSUM") as mmps,
            tc.tile_pool(name="sps", bufs=2, space="PSUM") as sps,
            tc.tile_pool(name="ptps", bufs=2, space="PSUM") as ptps,
            tc.tile_pool(name="otps", bufs=2, space="PSUM") as otps,
        ):
            # ---- constants into SBUF ----
            xt_t = cpool.tile([128, 3, T], BF16)
            for ct in range(3):
                nc.sync.dma_start(xt_t[:, ct, :], xt_d[128 * ct:128 * (ct + 1), :])
            wqk_t = cpool.tile([128, 4, 3, 128], BF16)
            for mt in range(4):
                for ct in range(3):
                    nc.sync.dma_start(wqk_t[:, mt, ct, :], wqk_d[mt, 128 * ct:128 * (ct + 1), :])
            wv_t = cpool.tile([128, 3, DH], BF16)
            for ct in range(3):
                nc.sync.dma_start(wv_t[:, ct, :], wv_d[128 * ct:128 * (ct + 1), :])
            wp_t = cpool.tile([128, NH, DIM], BF16)
            for kt in range(NH):
                nc.sync.dma_start(wp_t[:, kt, :], wp_d[128 * kt:128 * (kt + 1), :])
            eb_t = cpool.tile([128, NH, 392], BF16)
            nc.sync.dma_start(eb_t[:], eb_d[:])
            bqk_t = cpool.tile([128, 4], F32)
            nc.sync.dma_start(bqk_t[:], bqk_d[:])
            vb_t = cpool.tile([128, DH], F32)
            nc.sync.dma_start(vb_t[:], vb_d[:])
            pb_t = cpool.tile([128, DIM], F32)
            nc.sync.dma_start(pb_t[:], pb_d[:])
            pbr_t = cpool.tile([1, DIM], BF16)
            nc.sync.dma_start(pbr_t[:], pbr_d[:])
            one_t = cpool.tile([1, 128], BF16)
            nc.sync.dma_start(one_t[:], one_d[:])
            id_t = cpool.tile([128, 128], BF16)
            nc.sync.dma_start(id_t[:], id_d[:])
            hc_t = cpool.tile([128, 1], F32)
            nc.sync.dma_start(hc_t[:], hc_d[:])

            # ---- qk^T phase: tQ/tK hold q/k of heads 0-3 at partition 32h;
            #      tQ2/tK2 hold heads 4-5 at partition 32(h-4) (rows 0:64) ----
            tQ = qpool.tile([128, T], BF16, tag="tq")
            tK = qpool.tile([128, T], BF16, tag="tk")
            tQ2 = qpool.tile([128, T], BF16, tag="tq2")
            tK2 = qpool.tile([128, T], BF16, tag="tk2")
            qk_dests = [(0, tQ, 128), (1, tK, 128), (2, tQ2, 64), (3, tK2, 64)]
            for ch in range(nch):
                c0 = 392 * ch
                for mt, dest, msz in qk_dests:
                    ps = mmps.tile([128, 392], F32, tag="mm")
                    for ct in range(3):
                        nc.tensor.matmul(
                            ps[0:msz, :],
                            wqk_t[:, mt, ct, 0:msz],
                            xt_t[:, ct, c0:c0 + 392],
                            start=(ct == 0), stop=(ct == 2),
                        )
                    nc.scalar.activation(
                        dest[0:msz, c0:c0 + 392], ps[0:msz, :], AF.Identity,
                        bias=bqk_t[0:msz, mt:mt + 1], scale=1.0,
                    )

            for b in range(bc):
                b0 = b * N
                # ---- v natural [tokens, 768] for this batch ----
                v_t = vpool.tile([128, 2, DH], BF16, tag="v")
                for nt, (r0, nsz) in enumerate(NT):
                    for half in range(2):
                        h0 = 384 * half
                        ps = mmps.tile([128, 392], F32, tag="mm")
                        for ct in range(3):
                            nc.tensor.matmul(
                                ps[0:nsz, 0:384],
                                xt_t[:, ct, b0 + r0:b0 + r0 + nsz],
                                wv_t[:, ct, h0:h0 + 384],
                                start=(ct == 0), stop=(ct == 2),
                            )
                        nc.vector.tensor_add(
                            v_t[0:nsz, nt, h0:h0 + 384], ps[0:nsz, 0:384],
                            vb_t[0:nsz, h0:h0 + 384],
                        )

                if STAGES < 2:
                    continue
                # ---- stage 1 per head: S, exp, *exp(bias) with fused
                #      row-sum -> den ----
                den = denpool.tile([128, 2 * NH], F32, tag="den")
                nc.gpsimd.memset(den[:], 1.0)
                p_bs = []
                for h in range(NH):
                    if h < 4:
                        qsrc, ksrc, base = tQ, tK, 32 * h
                    else:
                        qsrc, ksrc, base = tQ2, tK2, 32 * (h - 4)
                    s_ps = sps.tile([128, 392], F32, tag="s")
                    for nt, (r0, nsz) in enumerate(NT):
                        nc.tensor.matmul(
                            s_ps[0:nsz, 196 * nt:196 * nt + 196],
                            qsrc[base:base + 32, b0 + r0:b0 + r0 + nsz],
                            ksrc[base:base + 32, b0:b0 + 196],
                            start=True, stop=True,
                            tile_position=(base, 0),
                        )
                    p_u = pupool.tile([128, 392], BF16, tag="pu", name=f"pu{h}")
                    if EXACT_RANGES:
                        for nt, (r0, nsz) in enumerate(NT):
                            reg = slice(196 * nt, 196 * nt + 196)
                            nc.scalar.activation(p_u[0:nsz, reg], s_ps[0:nsz, reg], AF.Exp)
                    else:
                        nc.scalar.activation(p_u[:, :], s_ps[:, :], AF.Exp)
                    p_b = pbpool.tile([128, 392], BF16, tag="pb", name=f"pb{h}")
                    for nt, (r0, nsz) in enumerate(NT):
                        reg = slice(196 * nt, 196 * nt + 196)
                        nc.vector.scalar_tensor_tensor(
                            out=p_b[0:nsz, reg], in0=p_u[0:nsz, reg], scalar=0.0,
                            in1=eb_t[0:nsz, h, reg], op0=OP.bypass, op1=OP.mult,
                            accum_out=den[0:nsz, 2 * h + nt:2 * h + nt + 1],
                        )
                    p_bs.append(p_b)
                rho = denpool.tile([128, 2 * NH], F32, tag="rho")
                nc.vector.reciprocal(rho[:, :], den[:, :])

                # ---- stage 2: diag + P.T per head; O.T packed two heads
                #      per PSUM bank so hardswish runs once per head-pair ----
                if STAGES < 3:
                    continue
                at_b = atpool.tile([128, NH * 196], BF16, tag="at")
                for hp in range(NH // 2):
                    ot_ps = otps.tile([128, 392], F32, tag="ot")
                    for hh in range(2):
                        h = 2 * hp + hh
                        p_b = p_bs[h]
                        dg = dgpool.tile([128, 196], BF16, tag="dg")
                        nc.gpsimd.tensor_scalar_mul(
                            dg[0:128, 0:128], id_t[:, :], rho[0:128, 2 * h:2 * h + 1])
                        nc.gpsimd.tensor_scalar_mul(
                            dg[0:68, 128:196], id_t[0:68, 0:68], rho[0:68, 2 * h + 1:2 * h + 2])
                        pt_ps = ptps.tile([128, 392], F32, tag="pt")
                        for mt, (m0, msz) in enumerate(NT):
                            for nt, (n0, nsz) in enumerate(NT):
                                nc.tensor.matmul(
                                    pt_ps[0:msz, 196 * mt + n0:196 * mt + n0 + nsz],
                                    p_b[0:nsz, 196 * nt + m0:196 * nt + m0 + msz],
                                    dg[0:nsz, 128 * nt:128 * nt + nsz],
                                    start=True, stop=True,
                                )
                        if STAGES < 4:
                            continue
                        pt_sb = ptspool.tile([128, 392], BF16, tag="pts")
                        if EXACT_RANGES:
                            nc.scalar.activation(pt_sb[0:128, 0:196], pt_ps[0:128, 0:196], AF.Copy)
                            nc.scalar.activation(pt_sb[0:68, 196:392], pt_ps[0:68, 196:392], AF.Copy)
                        else:
                            nc.scalar.activation(pt_sb[:, :], pt_ps[:, :], AF.Copy)
                        # ---- O.T [128, 196] = v.T @ P.T ----
                        for kt, (k0, ksz) in enumerate(NT):
                            nc.tensor.matmul(
                                ot_ps[:, 196 * hh:196 * hh + 196],
                                v_t[0:ksz, kt, 128 * h:128 * h + 128],
                                pt_sb[0:ksz, 196 * kt:196 * kt + 196],
                                start=(kt == 0), stop=(kt == 1),
                            )
                    if STAGES < 4:
                        continue
                    # ---- 6*hardswish(O) = O*(clamp(O,-3,3)+3) per pair;
                    #      the /6 is folded into the projection weights ----
                    m_t = rpool.tile([128, 392], BF16, tag="m")
                    nc.vector.tensor_scalar(
                        out=m_t[:, :], in0=ot_ps[:, :],
                        scalar1=3.0, scalar2=-3.0, op0=OP.min, op1=OP.max,
                    )
                    nc.vector.scalar_tensor_tensor(
                        out=at_b[:, 392 * hp:392 * hp + 392], in0=m_t[:, :], scalar=3.0,
                        in1=ot_ps[:, :], op0=OP.add, op1=OP.mult,
                    )

                # ---- proj for this batch ----
                if STAGES < 5:
                    continue
                for nt, (r0, nsz) in enumerate(NT):
                    ps = mmps.tile([128, 392], F32, tag="mm")
                    nc.tensor.matmul(
                        ps[0:nsz, 0:384], one_t[0:1, 0:nsz], pbr_t[0:1, :],
                        start=True, stop=False,
                    )
                    for kt in range(NH):
                        nc.tensor.matmul(
                            ps[0:nsz, 0:384],
                            at_b[:, 196 * kt + r0:196 * kt + r0 + nsz], wp_t[:, kt, :],
                            start=False, stop=(kt == NH - 1),
                        )
                    ob = opool.tile([128, DIM], F32, tag="ob")
                    nc.scalar.activation(ob[0:nsz, :], ps[0:nsz, 0:384], AF.Copy)
                    nc.sync.dma_start(out_d[b0 + r0:b0 + r0 + nsz, :], ob[0:nsz, :])

    nc.finalize()  # run Bacc passes (reg alloc, wait splitting) before walrus
    return nc


def _host_pack(x, qkv_w, qkv_b, proj_w, proj_b, attn_biases, bias_idxs, bc):
    """Build the common (replicated) input map and per-core xt slices."""
    w = np.asarray(qkv_w, np.float32).reshape(NH, 192, DIM)
    bia = np.asarray(qkv_b, np.float32).reshape(NH, 192)
    qw = w[:, 0:KD, :] * SCALE          # [6, 32, 384]
    kw = w[:, KD:2 * KD, :]
    vw = w[:, 2 * KD:, :]               # [6, 128, 384]
    qb = bia[:, 0:KD] * SCALE
    kb = bia[:, KD:2 * KD]
    vb = bia[:, 2 * KD:]

    wqk = np.zeros((4, DIM, 128), np.float32)
    wqk[0, :, :] = qw[0:4].reshape(128, DIM).T
    wqk[1, :, :] = kw[0:4].reshape(128, DIM).T
    wqk[2, :, 0:64] = qw[4:6].reshape(64, DIM).T
    wqk[3, :, 0:64] = kw[4:6].reshape(64, DIM).T
    bqk = np.zeros((128, 4), np.float32)
    bqk[:, 0] = qb[0:4].reshape(128)
    bqk[:, 1] = kb[0:4].reshape(128)
    bqk[0:64, 2] = qb[4:6].reshape(64)
    bqk[0:64, 3] = kb[4:6].reshape(64)

    wv = vw.reshape(DH, DIM).T.copy()          # [384, 768], head h at cols 128h
    vbt = np.tile(vb.reshape(1, DH), (128, 1)).astype(np.float32)
    # device computes 6*hardswish; absorb the 1/6 into the projection weights
    wp = (np.asarray(proj_w, np.float32).T / 6.0).copy()  # [768, 384]
    pbt = np.tile(np.asarray(proj_b, np.float32).reshape(1, DIM), (128, 1))

    bmat = np.asarray(attn_biases, np.float32)[:, np.asarray(bias_idxs)]  # [6,196,196]
    ebp = np.zeros((128, NH, 392), np.float32)
    eb = np.exp(bmat)
    for h in range(NH):
        ebp[0:128, h, 0:196] = eb[h, 0:128, :]
        ebp[0:68, h, 196:392] = eb[h, 128:196, :]

    common = {
        "pbr": np.asarray(proj_b, np.float32).reshape(1, DIM).astype(NPBF16),
        "ones": np.ones((1, 128), NPBF16),
        "wqk": wqk.astype(NPBF16),
        "bqk": bqk,
        "wv": wv.astype(NPBF16),
        "vb": vbt,
        "wp": wp.astype(NPBF16),
        "pb": pbt,
        "eb": ebp.astype(NPBF16),
        "idm": np.eye(128, dtype=NPBF16),
        "hc": np.full((128, 1), 0.5, np.float32),
    }

    x = np.asarray(x, np.float32)
    n_cores = x.shape[0] // bc
    xts = []
    for c in range(n_cores):
        xc = x[bc * c:bc * (c + 1)].reshape(bc * N, DIM)
        xts.append(np.ascontiguousarray(xc.T).astype(NPBF16))
    return common, xts


_NC_CACHE = {}


def kernel(x, qkv_w, qkv_b, proj_w, proj_b, attn_biases, bias_idxs):
    bc = B // N_CORES
    if bc not in _NC_CACHE:
        _NC_CACHE[bc] = _build_nc(bc)
    nc = _NC_CACHE[bc]
    common, xts = _host_pack(x, qkv_w, qkv_b, proj_w, proj_b, attn_biases, bias_idxs, bc)
    in_maps = [dict(common, xt=xts[c]) for c in range(N_CORES)]
    trace = bool(int(os.environ.get("KT_TRACE", "0")))
    res = run_bass_kernel_spmd(nc, in_maps, list(range(N_CORES)), trace=trace)
    LAST_RESULT["exec_time_ns"] = res.exec_time_ns
    LAST_RESULT["mean_exec_time_ns"] = res.mean_exec_time_ns
    outs = [res.results[c]["out"].reshape(bc, N, DIM) for c in range(N_CORES)]
    return np.concatenate(outs, axis=0).astype(np.float32)



# revision 27
# speedup vs baseline: 20.8347x; 20.8347x over previous
"""LeViT-style attention block kernel for Trainium2 (8 NeuronCores, data-parallel over batch).

Reference computation (per batch b of 256, N=196 tokens, DIM=384):
  qkv = x @ qkv_w.T + qkv_b                      [196, 1152]
  q,k,v per head h (6): q,k [196,32], v [196,128]
  S = q @ k.T * 32^-0.5 + bias_h                 [196, 196]
  P = softmax(S, -1)
  O = P @ v  (concat heads -> [196, 768])
  A = hardswish(O)
  out = A @ proj_w.T + proj_b                    [196, 384]

Device mapping (per core: 32 batches = 6272 tokens):
  - host prepacks x.T (bf16), weight tiles, exp(bias) tables
  - qk computed transposed ([head-packed 128 rows, tokens]) on PE
  - v computed natural per batch ([tokens, 768])
  - S natural [n, m] -> exp on ACT -> *exp(bias) with fused row-sum on DVE
    -> P.T via PE matmul against diag(1/den) (transpose + normalize in one)
    -> O.T = v.T @ P.T on PE -> hardswish (ACT relu + fused min*mul on DVE)
    -> A.T accumulated into 128-token chunk tiles -> proj matmul -> out.
"""

import os
import sys

import numpy as np

sys.path.insert(0, "/opt/trn_rl_repo")

import ml_dtypes  # noqa: E402

import concourse.bass as bass  # noqa: E402
import concourse.tile as tile  # noqa: E402
from concourse import bacc, mybir  # noqa: E402
from concourse.bass_utils import run_bass_kernel_spmd  # noqa: E402

BF16 = mybir.dt.bfloat16
F32 = mybir.dt.float32
NPBF16 = ml_dtypes.bfloat16

N_CORES = 8
B, N, DIM = 256, 196, 384
KD, NH, D = 32, 6, 128  # key dim, heads, per-head v dim
DH = D * NH  # 768
RES = 14
SCALE = KD ** -0.5

AF = mybir.ActivationFunctionType
OP = mybir.AluOpType

# per-batch n/m tiling: token rows split 128 + 68
NT = [(0, 128), (128, 68)]

LAST_RESULT = {}  # test harness peeks at timing info here

# CoreSim rejects reads of never-written PSUM regions; the merged single-call
# exp / P.T-copy read (and discard) such garbage. EXACT_RANGES=True emits
# per-region calls instead — numerically identical — for simulator validation.
EXACT_RANGES = False
STAGES = 5  # debug bisect: 1=qkv only, 2=+S/exp/ttr, 3=+diag/PT, 4=+AV/hswish, 5=full


def _build_nc(bc, repeat=1):
    """Build the single-core program for bc batches (bc*196 tokens).

    repeat>1 wraps the whole compute body in a hardware loop that re-runs
    it `repeat` times on the same inputs (used by timing harnesses to
    amortize fixed dispatch overhead; outputs are identical each pass).
    """
    T = bc * N
    assert T % 392 == 0
    nch = T // 392

    nc = bacc.Bacc("TRN2", target_bir_lowering=False, debug=False, num_devices=N_CORES)

    xt_d = nc.dram_tensor("xt", [DIM, T], BF16, kind="ExternalInput")
    wqk_d = nc.dram_tensor("wqk", [4, DIM, 128], BF16, kind="ExternalInput")
    bqk_d = nc.dram_tensor("bqk", [128, 4], F32, kind="ExternalInput")
    wv_d = nc.dram_tensor("wv", [DIM, DH], BF16, kind="ExternalInput")
    vb_d = nc.dram_tensor("vb", [128, DH], F32, kind="ExternalInput")
    wp_d = nc.dram_tensor("wp", [DH, DIM], BF16, kind="ExternalInput")
    pb_d = nc.dram_tensor("pb", [128, DIM], F32, kind="ExternalInput")
    pbr_d = nc.dram_tensor("pbr", [1, DIM], BF16, kind="ExternalInput")
    one_d = nc.dram_tensor("ones", [1, 128], BF16, kind="ExternalInput")

    eb_d = nc.dram_tensor("eb", [128, NH, 392], BF16, kind="ExternalInput")
    id_d = nc.dram_tensor("idm", [128, 128], BF16, kind="ExternalInput")
    out_d = nc.dram_tensor("out", [T, DIM], F32, kind="ExternalOutput")

    with tile.TileContext(nc) as tc:
        with (
            tc.tile_pool(name="const", bufs=1) as cpool,
            tc.tile_pool(name="qkt", bufs=1) as qpool,
            tc.tile_pool(name="vtile", bufs=6) as vpool,
            tc.tile_pool(name="pu", bufs=3) as pupool,
            tc.tile_pool(name="pb2", bufs=8) as pbpool,
            tc.tile_pool(name="pts", bufs=3) as ptspool,
            tc.tile_pool(name="dg", bufs=3) as dgpool,
            tc.tile_pool(name="den", bufs=6) as denpool,
            tc.tile_pool(name="rt", bufs=3) as rpool,
            tc.tile_pool(name="atc", bufs=2) as atpool,
            tc.tile_pool(name="ob", bufs=3) as opool,
            tc.tile_pool(name="mmps", bufs=2, space="PSUM") as mmps,
            tc.tile_pool(name="sps", bufs=2, space="PSUM") as sps,
            tc.tile_pool(name="ptps", bufs=2, space="PSUM") as ptps,
            tc.tile_pool(name="otps", bufs=2, space="PSUM") as otps,
        ):
            # ---- constants into SBUF ----
            # order: small weights first, then xt in token chunks, so the
            # first qkv matmuls can start within a few us
            wqk_t = cpool.tile([128, 4, 3, 128], BF16)
            for mt in range(4):
                for ct in range(3):
                    nc.sync.dma_start(wqk_t[:, mt, ct, :], wqk_d[mt, 128 * ct:128 * (ct + 1), :])
            bqk_t = cpool.tile([128, 4], F32)
            nc.sync.dma_start(bqk_t[:], bqk_d[:])
            wv_t = cpool.tile([128, 3, DH], BF16)
            for ct in range(3):
                nc.sync.dma_start(wv_t[:, ct, :], wv_d[128 * ct:128 * (ct + 1), :])
            vb_t = cpool.tile([128, DH], F32)
            nc.sync.dma_start(vb_t[:], vb_d[:])
            eb_t = cpool.tile([128, NH, 392], BF16)
            nc.sync.dma_start(eb_t[:], eb_d[:])
            wp_t = cpool.tile([128, NH, DIM], BF16)
            for kt in range(NH):
                nc.sync.dma_start(wp_t[:, kt, :], wp_d[128 * kt:128 * (kt + 1), :])
            pb_t = cpool.tile([128, DIM], F32)
            nc.sync.dma_start(pb_t[:], pb_d[:])
            pbr_t = cpool.tile([1, DIM], BF16)
            nc.sync.dma_start(pbr_t[:], pbr_d[:])
            one_t = cpool.tile([1, 128], BF16)
            nc.sync.dma_start(one_t[:], one_d[:])
            id_t = cpool.tile([128, 128], BF16)
            nc.sync.dma_start(id_t[:], id_d[:])
            xt_t = cpool.tile([128, 3, T], BF16)
            for ch in range(nch):
                c0 = 392 * ch
                for ct in range(3):
                    nc.sync.dma_start(
                        xt_t[:, ct, c0:c0 + 392],
                        xt_d[128 * ct:128 * (ct + 1), c0:c0 + 392],
                    )

            pools = (qpool, vpool, pupool, pbpool, ptspool, dgpool, denpool,
                     rpool, atpool, opool, mmps, sps, ptps, otps)
            consts = (xt_t, wqk_t, wv_t, wp_t, eb_t, bqk_t, vb_t, pb_t, id_t,
                      pbr_t, one_t, out_d)
            rep_ctx = tc.For_i(0, repeat) if repeat > 1 else _nullctx()
            with rep_ctx:
                _emit_body(nc, tc, bc, nch, pools, consts)
    nc.finalize()  # run Bacc passes (reg alloc, wait splitting) before walrus
    return nc


class _nullctx:
    def __enter__(self):
        return None

    def __exit__(self, *a):
        return False


def _emit_body(nc, tc, bc, nch, pools, consts):
    T = bc * N
    (qpool, vpool, pupool, pbpool, ptspool, dgpool, denpool, rpool, atpool,
     opool, mmps, sps, ptps, otps) = pools
    (xt_t, wqk_t, wv_t, wp_t, eb_t, bqk_t, vb_t, pb_t, id_t, pbr_t, one_t,
     out_d) = consts
    if True:
            # ---- qk^T phase: tQ/tK hold q/k of heads 0-3 at partition 32h;
            #      tQ2/tK2 hold heads 4-5 at partition 32(h-4) (rows 0:64) ----
            tQ = qpool.tile([128, T], BF16, tag="tq")
            tK = qpool.tile([128, T], BF16, tag="tk")
            tQ2 = qpool.tile([128, T], BF16, tag="tq2")
            tK2 = qpool.tile([128, T], BF16, tag="tk2")
            qk_dests = [(0, tQ, 128), (1, tK, 128), (2, tQ2, 64), (3, tK2, 64)]
            for ch in range(nch):
                c0 = 392 * ch
                for mt, dest, msz in qk_dests:
                    ps = mmps.tile([128, 392], F32, tag="mm")
                    for ct in range(3):
                        nc.tensor.matmul(
                            ps[0:msz, :],
                            wqk_t[:, mt, ct, 0:msz],
                            xt_t[:, ct, c0:c0 + 392],
                            start=(ct == 0), stop=(ct == 2),
                        )
                    nc.scalar.activation(
                        dest[0:msz, c0:c0 + 392], ps[0:msz, :], AF.Identity,
                        bias=bqk_t[0:msz, mt:mt + 1], scale=1.0,
                    )

            G = 1  # proj runs per G-batch group
            for b in range(bc):
                b0 = b * N
                gb0 = (b % G) * N
                if b % G == 0:
                    at_g = atpool.tile([128, NH, G * N], BF16, tag="at")
                # ---- v natural [tokens, 768] for this batch ----
                v_t = vpool.tile([128, 2, DH], BF16, tag="v")
                for nt, (r0, nsz) in enumerate(NT):
                    for half in range(2):
                        h0 = 384 * half
                        ps = mmps.tile([128, 392], F32, tag="mm")
                        for ct in range(3):
                            nc.tensor.matmul(
                                ps[0:nsz, 0:384],
                                xt_t[:, ct, b0 + r0:b0 + r0 + nsz],
                                wv_t[:, ct, h0:h0 + 384],
                                start=(ct == 0), stop=(ct == 2),
                            )
                        nc.vector.tensor_add(
                            v_t[0:nsz, nt, h0:h0 + 384], ps[0:nsz, 0:384],
                            vb_t[0:nsz, h0:h0 + 384],
                        )

                if STAGES < 2:
                    continue
                # ---- stage 1 per head: S, exp, *exp(bias) with fused
                #      row-sum -> den ----
                den = denpool.tile([128, 2 * NH], F32, tag="den")
                nc.gpsimd.memset(den[:], 1.0)
                p_bs = []
                for h in range(NH):
                    if h < 4:
                        qsrc, ksrc, base = tQ, tK, 32 * h
                    else:
                        qsrc, ksrc, base = tQ2, tK2, 32 * (h - 4)
                    s_ps = sps.tile([128, 392], F32, tag="s")
                    for nt, (r0, nsz) in enumerate(NT):
                        nc.tensor.matmul(
                            s_ps[0:nsz, 196 * nt:196 * nt + 196],
                            qsrc[base:base + 32, b0 + r0:b0 + r0 + nsz],
                            ksrc[base:base + 32, b0:b0 + 196],
                            start=True, stop=True,
                            tile_position=(base, 0),
                        )
                    p_u = pupool.tile([128, 392], BF16, tag="pu", name=f"pu{h}")
                    if EXACT_RANGES:
                        for nt, (r0, nsz) in enumerate(NT):
                            reg = slice(196 * nt, 196 * nt + 196)
                            nc.scalar.activation(p_u[0:nsz, reg], s_ps[0:nsz, reg], AF.Exp)
                    else:
                        nc.scalar.activation(p_u[:, :], s_ps[:, :], AF.Exp)
                    p_b = pbpool.tile([128, 392], BF16, tag="pb", name=f"pb{h}")
                    for nt, (r0, nsz) in enumerate(NT):
                        reg = slice(196 * nt, 196 * nt + 196)
                        nc.vector.scalar_tensor_tensor(
                            out=p_b[0:nsz, reg], in0=p_u[0:nsz, reg], scalar=0.0,
                            in1=eb_t[0:nsz, h, reg], op0=OP.bypass, op1=OP.mult,
                            accum_out=den[0:nsz, 2 * h + nt:2 * h + nt + 1],
                        )
                    p_bs.append(p_b)
                rho = denpool.tile([128, 2 * NH], F32, tag="rho")
                nc.vector.reciprocal(rho[:, :], den[:, :])

                # ---- stage 2: diag + P.T per head; O.T packed two heads
                #      per PSUM bank so hardswish runs once per head-pair ----
                if STAGES < 3:
                    continue
                for hp in range(NH // 2):
                    ot_ps = otps.tile([128, 392], F32, tag="ot")
                    for hh in range(2):
                        h = 2 * hp + hh
                        p_b = p_bs[h]
                        dg = dgpool.tile([128, 196], BF16, tag="dg")
                        nc.gpsimd.tensor_scalar_mul(
                            dg[0:128, 0:128], id_t[:, :], rho[0:128, 2 * h:2 * h + 1])
                        nc.gpsimd.tensor_scalar_mul(
                            dg[0:68, 128:196], id_t[0:68, 0:68], rho[0:68, 2 * h + 1:2 * h + 2])
                        pt_ps = ptps.tile([128, 392], F32, tag="pt")
                        for mt, (m0, msz) in enumerate(NT):
                            for nt, (n0, nsz) in enumerate(NT):
                                nc.tensor.matmul(
                                    pt_ps[0:msz, 196 * mt + n0:196 * mt + n0 + nsz],
                                    p_b[0:nsz, 196 * nt + m0:196 * nt + m0 + msz],
                                    dg[0:nsz, 128 * nt:128 * nt + nsz],
                                    start=True, stop=True,
                                )
                        if STAGES < 4:
                            continue
                        pt_sb = ptspool.tile([128, 392], BF16, tag="pts")
                        if EXACT_RANGES:
                            nc.scalar.activation(pt_sb[0:128, 0:196], pt_ps[0:128, 0:196], AF.Copy)
                            nc.scalar.activation(pt_sb[0:68, 196:392], pt_ps[0:68, 196:392], AF.Copy)
                        else:
                            nc.scalar.activation(pt_sb[:, :], pt_ps[:, :], AF.Copy)
                        # ---- O.T [128, 196] = v.T @ P.T ----
                        for kt, (k0, ksz) in enumerate(NT):
                            nc.tensor.matmul(
                                ot_ps[:, 196 * hh:196 * hh + 196],
                                v_t[0:ksz, kt, 128 * h:128 * h + 128],
                                pt_sb[0:ksz, 196 * kt:196 * kt + 196],
                                start=(kt == 0), stop=(kt == 1),
                            )
                    if STAGES < 4:
                        continue
                    # ---- 6*hardswish(O) = O*(clamp(O,-3,3)+3) per pair;
                    #      the /6 is folded into the projection weights ----
                    m_t = rpool.tile([128, 392], BF16, tag="m")
                    nc.vector.tensor_scalar(
                        out=m_t[:, :], in0=ot_ps[:, :],
                        scalar1=3.0, scalar2=-3.0, op0=OP.min, op1=OP.max,
                    )
                    nc.vector.scalar_tensor_tensor(
                        out=at_g[:, 2 * hp:2 * hp + 2, gb0:gb0 + 196],
                        in0=m_t[:, :], scalar=3.0,
                        in1=ot_ps[:, :], op0=OP.add, op1=OP.mult,
                    )

                # ---- proj once per G-batch group, on full 128-token tiles ----
                if STAGES < 5:
                    continue
                if b % G == G - 1:
                    TG = G * N
                    g0 = (b - G + 1) * N
                    for r0 in range(0, TG, 128):
                        rsz = min(128, TG - r0)
                        ps = mmps.tile([128, 392], F32, tag="mm")
                        nc.tensor.matmul(
                            ps[0:rsz, 0:384], one_t[0:1, 0:rsz], pbr_t[0:1, :],
                            start=True, stop=False,
                        )
                        for kt in range(NH):
                            nc.tensor.matmul(
                                ps[0:rsz, 0:384],
                                at_g[:, kt, r0:r0 + rsz], wp_t[:, kt, :],
                                start=False, stop=(kt == NH - 1),
                            )
                        ob = opool.tile([128, DIM], F32, tag="ob")
                        nc.scalar.activation(ob[0:rsz, :], ps[0:rsz, 0:384], AF.Copy)
                        nc.sync.dma_start(out_d[g0 + r0:g0 + r0 + rsz, :], ob[0:rsz, :])


def _host_pack(x, qkv_w, qkv_b, proj_w, proj_b, attn_biases, bias_idxs, bc):
    """Build the common (replicated) input map and per-core xt slices."""
    w = np.asarray(qkv_w, np.float32).reshape(NH, 192, DIM)
    bia = np.asarray(qkv_b, np.float32).reshape(NH, 192)
    qw = w[:, 0:KD, :] * SCALE          # [6, 32, 384]
    kw = w[:, KD:2 * KD, :]
    vw = w[:, 2 * KD:, :]               # [6, 128, 384]
    qb = bia[:, 0:KD] * SCALE
    kb = bia[:, KD:2 * KD]
    vb = bia[:, 2 * KD:]

    wqk = np.zeros((4, DIM, 128), np.float32)
    wqk[0, :, :] = qw[0:4].reshape(128, DIM).T
    wqk[1, :, :] = kw[0:4].reshape(128, DIM).T
    wqk[2, :, 0:64] = qw[4:6].reshape(64, DIM).T
    wqk[3, :, 0:64] = kw[4:6].reshape(64, DIM).T
    bqk = np.zeros((128, 4), np.float32)
    bqk[:, 0] = qb[0:4].reshape(128)
    bqk[:, 1] = kb[0:4].reshape(128)
    bqk[0:64, 2] = qb[4:6].reshape(64)
    bqk[0:64, 3] = kb[4:6].reshape(64)

    wv = vw.reshape(DH, DIM).T.copy()          # [384, 768], head h at cols 128h
    vbt = np.tile(vb.reshape(1, DH), (128, 1)).astype(np.float32)
    # device computes 6*hardswish; absorb the 1/6 into the projection weights
    wp = (np.asarray(proj_w, np.float32).T / 6.0).copy()  # [768, 384]
    pbt = np.tile(np.asarray(proj_b, np.float32).reshape(1, DIM), (128, 1))

    bmat = np.asarray(attn_biases, np.float32)[:, np.asarray(bias_idxs)]  # [6,196,196]
    ebp = np.zeros((128, NH, 392), np.float32)
    eb = np.exp(bmat)
    for h in range(NH):
        ebp[0:128, h, 0:196] = eb[h, 0:128, :]
        ebp[0:68, h, 196:392] = eb[h, 128:196, :]

    common = {
        "pbr": np.asarray(proj_b, np.float32).reshape(1, DIM).astype(NPBF16),
        "ones": np.ones((1, 128), NPBF16),
        "wqk": wqk.astype(NPBF16),
        "bqk": bqk,
        "wv": wv.astype(NPBF16),
        "vb": vbt,
        "wp": wp.astype(NPBF16),
        "pb": pbt,
        "eb": ebp.astype(NPBF16),
        "idm": np.eye(128, dtype=NPBF16),
    }

    x = np.asarray(x, np.float32)
    n_cores = x.shape[0] // bc
    xts = []
    for c in range(n_cores):
        xc = x[bc * c:bc * (c + 1)].reshape(bc * N, DIM)
        xts.append(np.ascontiguousarray(xc.T).astype(NPBF16))
    return common, xts


_NC_CACHE = {}


def kernel(x, qkv_w, qkv_b, proj_w, proj_b, attn_biases, bias_idxs):
    bc = B // N_CORES
    if bc not in _NC_CACHE:
        _NC_CACHE[bc] = _build_nc(bc)
    nc = _NC_CACHE[bc]
    common, xts = _host_pack(x, qkv_w, qkv_b, proj_w, proj_b, attn_biases, bias_idxs, bc)
    in_maps = [dict(common, xt=xts[c]) for c in range(N_CORES)]
    trace = bool(int(os.environ.get("KT_TRACE", "0")))
    res = run_bass_kernel_spmd(nc, in_maps, list(range(N_CORES)), trace=trace)
    LAST_RESULT["exec_time_ns"] = res.exec_time_ns
    LAST_RESULT["mean_exec_time_ns"] = res.mean_exec_time_ns
    outs = [res.results[c]["out"].reshape(bc, N, DIM) for c in range(N_CORES)]
    return np.concatenate(outs, axis=0).astype(np.float32)



# revision 31
# speedup vs baseline: 72.8167x; 3.4950x over previous
"""LeViT-style attention block kernel for Trainium2 (8 NeuronCores, data-parallel over batch).

Reference computation (per batch b of 256, N=196 tokens, DIM=384):
  qkv = x @ qkv_w.T + qkv_b                      [196, 1152]
  q,k,v per head h (6): q,k [196,32], v [196,128]
  S = q @ k.T * 32^-0.5 + bias_h                 [196, 196]
  P = softmax(S, -1)
  O = P @ v  (concat heads -> [196, 768])
  A = hardswish(O)
  out = A @ proj_w.T + proj_b                    [196, 384]

Device mapping (per core: 32 batches = 6272 tokens):
  - host prepacks x.T (bf16), weight tiles, exp(bias) tables
  - qk computed transposed ([head-packed 128 rows, tokens]) on PE
  - v computed natural per batch ([tokens, 768])
  - S natural [n, m] -> exp on ACT -> *exp(bias) with fused row-sum on DVE
    -> P.T via PE matmul against diag(1/den) (transpose + normalize in one)
    -> O.T = v.T @ P.T on PE -> hardswish (ACT relu + fused min*mul on DVE)
    -> A.T accumulated into 128-token chunk tiles -> proj matmul -> out.
"""

import os
import sys

import numpy as np

sys.path.insert(0, "/opt/trn_rl_repo")

import ml_dtypes  # noqa: E402

import concourse.bass as bass  # noqa: E402
import concourse.tile as tile  # noqa: E402
from concourse import bacc, mybir  # noqa: E402
from concourse.bass_utils import run_bass_kernel_spmd  # noqa: E402

BF16 = mybir.dt.bfloat16
F32 = mybir.dt.float32
NPBF16 = ml_dtypes.bfloat16

N_CORES = 8
B, N, DIM = 256, 196, 384
KD, NH, D = 32, 6, 128  # key dim, heads, per-head v dim
DH = D * NH  # 768
RES = 14
SCALE = KD ** -0.5

AF = mybir.ActivationFunctionType
OP = mybir.AluOpType

# per-batch n/m tiling: token rows split 128 + 68
NT = [(0, 128), (128, 68)]

LAST_RESULT = {}  # test harness peeks at timing info here

# CoreSim rejects reads of never-written PSUM regions; the merged single-call
# exp / P.T-copy read (and discard) such garbage. EXACT_RANGES=True emits
# per-region calls instead — numerically identical — for simulator validation.
EXACT_RANGES = False
STAGES = 5  # debug bisect: 1=qkv only, 2=+S/exp/ttr, 3=+diag/PT, 4=+AV/hswish, 5=full


def _build_nc(bc, repeat=1):
    """Build the single-core program for bc batches (bc*196 tokens).

    repeat>1 wraps the whole compute body in a hardware loop that re-runs
    it `repeat` times on the same inputs (used by timing harnesses to
    amortize fixed dispatch overhead; outputs are identical each pass).
    """
    T = bc * N
    assert T % 392 == 0
    nch = T // 392

    nc = bacc.Bacc("TRN2", target_bir_lowering=False, debug=False, num_devices=N_CORES)

    xt_d = nc.dram_tensor("xt", [DIM, T], BF16, kind="ExternalInput")
    wqk_d = nc.dram_tensor("wqk", [4, DIM, 128], BF16, kind="ExternalInput")
    bqk_d = nc.dram_tensor("bqk", [128, 4], F32, kind="ExternalInput")
    wv_d = nc.dram_tensor("wv", [DIM, DH], BF16, kind="ExternalInput")
    vb_d = nc.dram_tensor("vb", [128, DH], F32, kind="ExternalInput")
    wp_d = nc.dram_tensor("wp", [DH, DIM], BF16, kind="ExternalInput")
    pb_d = nc.dram_tensor("pb", [128, DIM], F32, kind="ExternalInput")
    pbr_d = nc.dram_tensor("pbr", [1, DIM], BF16, kind="ExternalInput")
    one_d = nc.dram_tensor("ones", [1, 128], BF16, kind="ExternalInput")

    eb_d = nc.dram_tensor("eb", [128, NH, 392], BF16, kind="ExternalInput")
    id_d = nc.dram_tensor("idm", [128, 128], BF16, kind="ExternalInput")
    out_d = nc.dram_tensor("out", [T, DIM], F32, kind="ExternalOutput")

    with tile.TileContext(nc) as tc:
        with (
            tc.tile_pool(name="const", bufs=1) as cpool,
            tc.tile_pool(name="qkt", bufs=1) as qpool,
            tc.tile_pool(name="vtile", bufs=6) as vpool,
            tc.tile_pool(name="pu", bufs=3) as pupool,
            tc.tile_pool(name="pb2", bufs=8) as pbpool,
            tc.tile_pool(name="pts", bufs=3) as ptspool,
            tc.tile_pool(name="dg", bufs=3) as dgpool,
            tc.tile_pool(name="den", bufs=6) as denpool,
            tc.tile_pool(name="rt", bufs=3) as rpool,
            tc.tile_pool(name="atc", bufs=2) as atpool,
            tc.tile_pool(name="ob", bufs=3) as opool,
            tc.tile_pool(name="mmps", bufs=2, space="PSUM") as mmps,
            tc.tile_pool(name="sps", bufs=2, space="PSUM") as sps,
            tc.tile_pool(name="ptps", bufs=2, space="PSUM") as ptps,
            tc.tile_pool(name="otps", bufs=2, space="PSUM") as otps,
        ):
            # ---- constants into SBUF ----
            # order: small weights first, then xt in token chunks, so the
            # first qkv matmuls can start within a few us
            wqk_t = cpool.tile([128, 4, 3, 128], BF16)
            for mt in range(4):
                for ct in range(3):
                    nc.sync.dma_start(wqk_t[:, mt, ct, :], wqk_d[mt, 128 * ct:128 * (ct + 1), :])
            bqk_t = cpool.tile([128, 4], F32)
            nc.sync.dma_start(bqk_t[:], bqk_d[:])
            wv_t = cpool.tile([128, 3, DH], BF16)
            for ct in range(3):
                nc.sync.dma_start(wv_t[:, ct, :], wv_d[128 * ct:128 * (ct + 1), :])
            vb_t = cpool.tile([128, DH], F32)
            nc.sync.dma_start(vb_t[:], vb_d[:])
            eb_t = cpool.tile([128, NH, 392], BF16)
            nc.sync.dma_start(eb_t[:], eb_d[:])
            wp_t = cpool.tile([128, NH, DIM], BF16)
            for kt in range(NH):
                nc.sync.dma_start(wp_t[:, kt, :], wp_d[128 * kt:128 * (kt + 1), :])
            pb_t = cpool.tile([128, DIM], F32)
            nc.sync.dma_start(pb_t[:], pb_d[:])
            pbr_t = cpool.tile([1, DIM], BF16)
            nc.sync.dma_start(pbr_t[:], pbr_d[:])
            one_t = cpool.tile([1, 128], BF16)
            nc.sync.dma_start(one_t[:], one_d[:])
            id_t = cpool.tile([128, 128], BF16)
            nc.sync.dma_start(id_t[:], id_d[:])
            xt_t = cpool.tile([128, 3, T], BF16)
            for ch in range(nch):
                c0 = 392 * ch
                for ct in range(3):
                    nc.sync.dma_start(
                        xt_t[:, ct, c0:c0 + 392],
                        xt_d[128 * ct:128 * (ct + 1), c0:c0 + 392],
                    )

            pools = (qpool, vpool, pupool, pbpool, ptspool, dgpool, denpool,
                     rpool, atpool, opool, mmps, sps, ptps, otps)
            consts = (xt_t, wqk_t, wv_t, wp_t, eb_t, bqk_t, vb_t, pb_t, id_t,
                      pbr_t, one_t, out_d)
            rep_ctx = tc.For_i(0, repeat) if repeat > 1 else _nullctx()
            with rep_ctx:
                _emit_body(nc, tc, bc, nch, pools, consts)
    nc.finalize()  # run Bacc passes (reg alloc, wait splitting) before walrus
    return nc


class _nullctx:
    def __enter__(self):
        return None

    def __exit__(self, *a):
        return False


def _emit_body(nc, tc, bc, nch, pools, consts):
    T = bc * N
    (qpool, vpool, pupool, pbpool, ptspool, dgpool, denpool, rpool, atpool,
     opool, mmps, sps, ptps, otps) = pools
    (xt_t, wqk_t, wv_t, wp_t, eb_t, bqk_t, vb_t, pb_t, id_t, pbr_t, one_t,
     out_d) = consts
    if True:
            # ---- qk^T phase: tQ/tK hold q/k of heads 0-3 at partition 32h;
            #      tQ2/tK2 hold heads 4-5 at partition 32(h-4) (rows 0:64) ----
            tQ = qpool.tile([128, T], BF16, tag="tq")
            tK = qpool.tile([128, T], BF16, tag="tk")
            tQ2 = qpool.tile([128, T], BF16, tag="tq2")
            tK2 = qpool.tile([128, T], BF16, tag="tk2")
            qk_dests = [(0, tQ, 128), (1, tK, 128), (2, tQ2, 64), (3, tK2, 64)]
            for ch in range(nch):
                c0 = 392 * ch
                for mt, dest, msz in qk_dests:
                    ps = mmps.tile([128, 392], F32, tag="mm")
                    for ct in range(3):
                        nc.tensor.matmul(
                            ps[0:msz, :],
                            wqk_t[:, mt, ct, 0:msz],
                            xt_t[:, ct, c0:c0 + 392],
                            start=(ct == 0), stop=(ct == 2),
                        )
                    nc.scalar.activation(
                        dest[0:msz, c0:c0 + 392], ps[0:msz, :], AF.Identity,
                        bias=bqk_t[0:msz, mt:mt + 1], scale=1.0,
                    )

            G = 1  # proj runs per G-batch group
            for b in range(bc):
                b0 = b * N
                gb0 = (b % G) * N
                if b % G == 0:
                    at_g = atpool.tile([128, NH, G * N], BF16, tag="at")
                # ---- v natural [tokens, 768] for this batch ----
                v_t = vpool.tile([128, 2, DH], BF16, tag="v")
                for nt, (r0, nsz) in enumerate(NT):
                    for half in range(2):
                        h0 = 384 * half
                        ps = mmps.tile([128, 392], F32, tag="mm")
                        for ct in range(3):
                            nc.tensor.matmul(
                                ps[0:nsz, 0:384],
                                xt_t[:, ct, b0 + r0:b0 + r0 + nsz],
                                wv_t[:, ct, h0:h0 + 384],
                                start=(ct == 0), stop=(ct == 2),
                            )
                        nc.vector.tensor_add(
                            v_t[0:nsz, nt, h0:h0 + 384], ps[0:nsz, 0:384],
                            vb_t[0:nsz, h0:h0 + 384],
                        )

                if STAGES < 2:
                    continue
                # ---- stage 1 per head: S, exp, *exp(bias) with fused
                #      row-sum -> den ----
                den = denpool.tile([128, 2 * NH], F32, tag="den")
                nc.gpsimd.memset(den[:], 1.0)
                p_bs = []
                for h in range(NH):
                    if h < 4:
                        qsrc, ksrc, base = tQ, tK, 32 * h
                    else:
                        qsrc, ksrc, base = tQ2, tK2, 32 * (h - 4)
                    s_ps = sps.tile([128, 392], F32, tag="s")
                    for nt, (r0, nsz) in enumerate(NT):
                        nc.tensor.matmul(
                            s_ps[0:nsz, 196 * nt:196 * nt + 196],
                            qsrc[base:base + 32, b0 + r0:b0 + r0 + nsz],
                            ksrc[base:base + 32, b0:b0 + 196],
                            start=True, stop=True,
                            tile_position=(base, 0),
                        )
                    p_u = pupool.tile([128, 392], BF16, tag="pu", name=f"pu{h}")
                    if EXACT_RANGES:
                        for nt, (r0, nsz) in enumerate(NT):
                            reg = slice(196 * nt, 196 * nt + 196)
                            nc.scalar.activation(p_u[0:nsz, reg], s_ps[0:nsz, reg], AF.Exp)
                    else:
                        nc.scalar.activation(p_u[:, :], s_ps[:, :], AF.Exp)
                    p_b = pbpool.tile([128, 392], BF16, tag="pb", name=f"pb{h}")
                    for nt, (r0, nsz) in enumerate(NT):
                        reg = slice(196 * nt, 196 * nt + 196)
                        nc.vector.scalar_tensor_tensor(
                            out=p_b[0:nsz, reg], in0=p_u[0:nsz, reg], scalar=0.0,
                            in1=eb_t[0:nsz, h, reg], op0=OP.bypass, op1=OP.mult,
                            accum_out=den[0:nsz, 2 * h + nt:2 * h + nt + 1],
                        )
                    p_bs.append(p_b)
                rho = denpool.tile([128, 2 * NH], F32, tag="rho")
                nc.vector.reciprocal(rho[:, :], den[:, :])

                # ---- stage 2: diag + P.T per head; O.T packed two heads
                #      per PSUM bank so hardswish runs once per head-pair ----
                if STAGES < 3:
                    continue
                for hp in range(NH // 2):
                    ot_ps = otps.tile([128, 392], F32, tag="ot")
                    for hh in range(2):
                        h = 2 * hp + hh
                        p_b = p_bs[h]
                        dg = dgpool.tile([128, 196], BF16, tag="dg")
                        nc.gpsimd.tensor_scalar_mul(
                            dg[0:128, 0:128], id_t[:, :], rho[0:128, 2 * h:2 * h + 1])
                        nc.gpsimd.tensor_scalar_mul(
                            dg[0:68, 128:196], id_t[0:68, 0:68], rho[0:68, 2 * h + 1:2 * h + 2])
                        pt_ps = ptps.tile([128, 392], F32, tag="pt")
                        for mt, (m0, msz) in enumerate(NT):
                            for nt, (n0, nsz) in enumerate(NT):
                                nc.tensor.matmul(
                                    pt_ps[0:msz, 196 * mt + n0:196 * mt + n0 + nsz],
                                    p_b[0:nsz, 196 * nt + m0:196 * nt + m0 + msz],
                                    dg[0:nsz, 128 * nt:128 * nt + nsz],
                                    start=True, stop=True,
                                )
                        if STAGES < 4:
                            continue
                        pt_sb = ptspool.tile([128, 392], BF16, tag="pts")
                        if EXACT_RANGES:
                            nc.scalar.activation(pt_sb[0:128, 0:196], pt_ps[0:128, 0:196], AF.Copy)
                            nc.scalar.activation(pt_sb[0:68, 196:392], pt_ps[0:68, 196:392], AF.Copy)
                        else:
                            nc.scalar.activation(pt_sb[:, :], pt_ps[:, :], AF.Copy)
                        # ---- O.T [128, 196] = v.T @ P.T ----
                        for kt, (k0, ksz) in enumerate(NT):
                            nc.tensor.matmul(
                                ot_ps[:, 196 * hh:196 * hh + 196],
                                v_t[0:ksz, kt, 128 * h:128 * h + 128],
                                pt_sb[0:ksz, 196 * kt:196 * kt + 196],
                                start=(kt == 0), stop=(kt == 1),
                            )
                    if STAGES < 4:
                        continue
                    # ---- 6*hardswish(O) = O*(clamp(O,-3,3)+3) per pair;
                    #      the /6 is folded into the projection weights ----
                    m_t = rpool.tile([128, 392], BF16, tag="m")
                    nc.vector.tensor_scalar(
                        out=m_t[:, :], in0=ot_ps[:, :],
                        scalar1=3.0, scalar2=-3.0, op0=OP.min, op1=OP.max,
                    )
                    nc.vector.scalar_tensor_tensor(
                        out=at_g[:, 2 * hp:2 * hp + 2, gb0:gb0 + 196],
                        in0=m_t[:, :], scalar=3.0,
                        in1=ot_ps[:, :], op0=OP.add, op1=OP.mult,
                    )

                # ---- proj once per G-batch group, on full 128-token tiles ----
                if STAGES < 5:
                    continue
                if b % G == G - 1:
                    TG = G * N
                    g0 = (b - G + 1) * N
                    for r0 in range(0, TG, 128):
                        rsz = min(128, TG - r0)
                        ps = mmps.tile([128, 392], F32, tag="mm")
                        nc.tensor.matmul(
                            ps[0:rsz, 0:384], one_t[0:1, 0:rsz], pbr_t[0:1, :],
                            start=True, stop=False,
                        )
                        for kt in range(NH):
                            nc.tensor.matmul(
                                ps[0:rsz, 0:384],
                                at_g[:, kt, r0:r0 + rsz], wp_t[:, kt, :],
                                start=False, stop=(kt == NH - 1),
                            )
                        ob = opool.tile([128, DIM], F32, tag="ob")
                        nc.scalar.activation(ob[0:rsz, :], ps[0:rsz, 0:384], AF.Copy)
                        nc.sync.dma_start(out_d[g0 + r0:g0 + r0 + rsz, :], ob[0:rsz, :])


def _host_pack(x, qkv_w, qkv_b, proj_w, proj_b, attn_biases, bias_idxs, bc):
    """Build the common (replicated) input map and per-core xt slices."""
    w = np.asarray(qkv_w, np.float32).reshape(NH, 192, DIM)
    bia = np.asarray(qkv_b, np.float32).reshape(NH, 192)
    qw = w[:, 0:KD, :] * SCALE          # [6, 32, 384]
    kw = w[:, KD:2 * KD, :]
    vw = w[:, 2 * KD:, :]               # [6, 128, 384]
    qb = bia[:, 0:KD] * SCALE
    kb = bia[:, KD:2 * KD]
    vb = bia[:, 2 * KD:]

    wqk = np.zeros((4, DIM, 128), np.float32)
    wqk[0, :, :] = qw[0:4].reshape(128, DIM).T
    wqk[1, :, :] = kw[0:4].reshape(128, DIM).T
    wqk[2, :, 0:64] = qw[4:6].reshape(64, DIM).T
    wqk[3, :, 0:64] = kw[4:6].reshape(64, DIM).T
    bqk = np.zeros((128, 4), np.float32)
    bqk[:, 0] = qb[0:4].reshape(128)
    bqk[:, 1] = kb[0:4].reshape(128)
    bqk[0:64, 2] = qb[4:6].reshape(64)
    bqk[0:64, 3] = kb[4:6].reshape(64)

    wv = vw.reshape(DH, DIM).T.copy()          # [384, 768], head h at cols 128h
    vbt = np.tile(vb.reshape(1, DH), (128, 1)).astype(np.float32)
    # device computes 6*hardswish; absorb the 1/6 into the projection weights
    wp = (np.asarray(proj_w, np.float32).T / 6.0).copy()  # [768, 384]
    pbt = np.tile(np.asarray(proj_b, np.float32).reshape(1, DIM), (128, 1))

    bmat = np.asarray(attn_biases, np.float32)[:, np.asarray(bias_idxs)]  # [6,196,196]
    ebp = np.zeros((128, NH, 392), np.float32)
    eb = np.exp(bmat)
    for h in range(NH):
        ebp[0:128, h, 0:196] = eb[h, 0:128, :]
        ebp[0:68, h, 196:392] = eb[h, 128:196, :]

    common = {
        "pbr": np.asarray(proj_b, np.float32).reshape(1, DIM).astype(NPBF16),
        "ones": np.ones((1, 128), NPBF16),
        "wqk": wqk.astype(NPBF16),
        "bqk": bqk,
        "wv": wv.astype(NPBF16),
        "vb": vbt,
        "wp": wp.astype(NPBF16),
        "pb": pbt,
        "eb": ebp.astype(NPBF16),
        "idm": np.eye(128, dtype=NPBF16),
    }

    x = np.asarray(x, np.float32)
    n_cores = x.shape[0] // bc
    xts = []
    for c in range(n_cores):
        xc = x[bc * c:bc * (c + 1)].reshape(bc * N, DIM)
        xts.append(np.ascontiguousarray(xc.T).astype(NPBF16))
    return common, xts


_NC_CACHE = {}


def kernel(x, qkv_w, qkv_b, proj_w, proj_b, attn_biases, bias_idxs):
    bc = B // N_CORES
    if bc not in _NC_CACHE:
        _NC_CACHE[bc] = _build_nc(bc)
    nc = _NC_CACHE[bc]
    common, xts = _host_pack(x, qkv_w, qkv_b, proj_w, proj_b, attn_biases, bias_idxs, bc)
    in_maps = [dict(common, xt=xts[c]) for c in range(N_CORES)]
    trace = bool(int(os.environ.get("KT_TRACE", "0")))
    res = run_bass_kernel_spmd(nc, in_maps, list(range(N_CORES)), trace=trace)
    LAST_RESULT["exec_time_ns"] = res.exec_time_ns
    LAST_RESULT["mean_exec_time_ns"] = res.mean_exec_time_ns
    outs = [res.results[c]["out"].reshape(bc, N, DIM) for c in range(N_CORES)]
    return np.concatenate(outs, axis=0).astype(np.float32)

